# revision 14
# baseline (speedup 1.0000x reference)
"""Blocked-FP8 linear (dequant + matmul + bias) on 8 Trainium2 NeuronCores.

Computation: out[b,s,o] = sum_i x[b,s,i] * (weight[o,i] * scale_inv[o//128, i//128]) + bias[o]
Shapes: x [2, 2048, 4096] f32, weight [4096, 4096] f32 (e4m3-quantized values),
        weight_scale_inv [32, 32] f32, bias [4096] f32 -> out [2, 2048, 4096] f32.

Sharding: 2-way over tokens x 4-way over out_features (colwise tensor-parallel,
no collectives). Each core computes a [2048 token, 1024 out] block as
out.T = W_deq @ X.T with K(=in_features) on the partition dim.

Speed scheme (v2): the PE streams the moving operand at 1 column/cycle, so a
bf16 [128k x 512tok] matmul costs ~216 ns regardless of dtype. The only 2x
lever is fp8 DoubleRow (both operands fp8e4): one instruction contracts TWO
128-k-blocks in one 1024-column sweep (~231 ns measured, 1.87x per block).
Per core, F=10 of the 32 k-blocks ride fp8 DoubleRow; the rest stay bf16.
Per-(ti,j) group: 5 DR + 22 bf16 matmuls into one PSUM bank.

Accuracy budget (rel tol 2e-2): an fp8 block pays w-requant (~2.6e-2 rel,
block-dependent via the scale mantissa) + x-fp8 (~2.65e-2) on its slice.
Three tricks keep the total at ~1.79e-2 (sim matches HW to 4 digits):
 1. weights are pre-scaled by 64 before e4m3 quantization (values ~0.02 sit
    in e4m3's denormal zone otherwise; x64 moves them to the normal range).
    All weights (bf16 too, exact there) carry the x64; the bias step computes
    out = (psum + 64*bias) * (1/64) in one fused DVE tensor_scalar.
 2. block selection: which k-blocks go fp8 is chosen per core from the
    actual absolute error cost T*||q8(64 w_blk)-64 w_blk||^2/4096 +
    ||x_err(k)||^2*||w_blk||^2/128, summed over the core's 8 j-rows.
 3. row clustering: the assignment of the 32 global output-row-blocks to the
    4 column-shards is chosen (greedy) so rows sharing low-cost k-blocks land
    in the same shard; the host remaps rows on assemble.

Schedule: startup is HBM-bound. Rings: sync=weights (fp8 j0 first so the
DR-first group order can start on ~190KB), scalar=xb bf16 panels (chunked,
ti>=1 gated behind the weight stream), vector=x8 fp8 panels (small, land
first), gpsimd=consts+outputs (even j; odd j outputs ride the vector ring).
The final (ti=3,j=7) group splits its bias-add across vector/gpsimd halves
and its output DMA across four rings to shorten the tail drain.
"""

import os
import sys

for _p in ("/opt/trn_rl_repo", "/root/.axon_site/_ro/trn_rl_repo"):
    if os.path.isdir(_p) and _p not in sys.path:
        sys.path.insert(0, _p)

import ml_dtypes
import numpy as np

import concourse.bass as bass  # noqa: F401  (registers AP machinery)
import concourse.tile as tile
from concourse import bacc, mybir
from concourse.bass_utils import run_bass_kernel_spmd
from concourse.tile import add_dep_helper

BLOCK = 128
B, S, IN, OUT = 2, 2048, 4096, 4096
N_CORES = 8
TB_SPLIT = 2            # token split
OB_SPLIT = 4            # out_features split (shard = clustered row group)
T_SH = B * S // TB_SPLIT    # 2048 tokens per core
O_SH = OUT // OB_SPLIT      # 1024 out features per core
KB = IN // BLOCK            # 32 k-blocks
JB = O_SH // BLOCK          # 8 local o-blocks
NT = 4                      # token tiles per core
TW = 512                    # tokens per tile (PSUM bank width in fp32)
F8N = 12                    # fp8 DoubleRow k-blocks per core (6 DR pairs)
NB = KB - F8N               # bf16 k-blocks
WS = 64.0                   # weight pre-scale (out of e4m3 denormal zone)
N_WARM = 512                # PE clock warm-up matmuls (fill the ~2.5us DMA-domain ramp)

_BF16 = ml_dtypes.bfloat16
_F8 = ml_dtypes.float8_e4m3   # TRN float8e4 (IEEE-ish, max 240)
_DR = mybir.MatmulPerfMode.DoubleRow

_compiled = None


def _build_program():
    nc = bacc.Bacc("TRN2", target_bir_lowering=False, debug=False,
                   num_devices=N_CORES)

    x8s = [nc.dram_tensor(f"x8_{ti}", [BLOCK, F8N, TW], mybir.dt.float8e4,
                          kind="ExternalInput") for ti in range(NT)]
    xbs = [nc.dram_tensor(f"xb_{ti}", [BLOCK, NB, TW], mybir.dt.bfloat16,
                          kind="ExternalInput") for ti in range(NT)]
    w8 = nc.dram_tensor("w8", [JB, BLOCK, F8N, BLOCK], mybir.dt.float8e4,
                        kind="ExternalInput")
    wb = nc.dram_tensor("wb", [JB, BLOCK, NB, BLOCK], mybir.dt.bfloat16,
                        kind="ExternalInput")
    # bias columns (x64, to fold into (psum + 64 b) * (1/64)), padded 16x;
    # final 16 cols hold the UNSCALED bias of the last j-block for the
    # scalar-engine activation (out = Identity(psum * 1/64 + b)) in the tail.
    bc = nc.dram_tensor("bc", [BLOCK, (JB + 1) * 16], mybir.dt.float32,
                        kind="ExternalInput")
    # contiguous per-(ti,j) tiles: strided 2KB-row writes into a [O_SH,T_SH]
    # matrix drain at ~60 GB/s; a contiguous 256KB block is much faster.
    out = nc.dram_tensor("out", [NT, JB, BLOCK, TW], mybir.dt.float32,
                         kind="ExternalOutput")

    out_ap = out.ap()
    inv = 1.0 / WS

    with tile.TileContext(nc) as tc:
        with (
            tc.tile_pool(name="consts", bufs=1) as consts,
            tc.tile_pool(name="w8pool", bufs=JB) as w8pool,
            tc.tile_pool(name="wbpool", bufs=JB) as wbpool,
            tc.tile_pool(name="x8pool", bufs=NT) as x8pool,
            tc.tile_pool(name="xbpool", bufs=NT) as xbpool,
            tc.tile_pool(name="opool", bufs=8) as opool,
            tc.tile_pool(name="apool", bufs=JB) as apool,
            tc.tile_pool(name="pspool", bufs=7, space="PSUM") as pspool,
            tc.tile_pool(name="warmps", bufs=1, space="PSUM") as warmps,
        ):
            # PE warm-up on a zeroed scratch tile: keeps the PE clock ramp
            # (HAM) running so the first real matmuls execute at full rate.
            warm = consts.tile([BLOCK, BLOCK], mybir.dt.bfloat16)
            nc.gpsimd.memset(warm[:], 0.0)
            ps_warm = warmps.tile([BLOCK, BLOCK], mybir.dt.float32)
            for _ in range(N_WARM):
                nc.tensor.matmul(ps_warm[:], warm[:], warm[:],
                                 start=True, stop=True)

            bc_t = consts.tile([BLOCK, (JB + 1) * 16], mybir.dt.float32)
            nc.gpsimd.dma_start(out=bc_t[:], in_=bc.ap())

            w8_ts = [w8pool.tile([BLOCK, F8N, BLOCK], mybir.dt.float8e4,
                                 name=f"w8_{j}", tag="w8") for j in range(JB)]
            wb_ts = [wbpool.tile([BLOCK, NB, BLOCK], mybir.dt.bfloat16,
                                 name=f"wb_{j}", tag="wb") for j in range(JB)]
            x8_ts = [x8pool.tile([BLOCK, F8N, TW], mybir.dt.float8e4,
                                 name=f"x8t_{ti}", tag="x8") for ti in range(NT)]
            xb_ts = [xbpool.tile([BLOCK, NB, TW], mybir.dt.bfloat16,
                                 name=f"xbt_{ti}", tag="xb") for ti in range(NT)]

            # Startup plan: tile-0 is processed in TWO passes per j-group —
            # a DR-only pass (all fp8: ~190KB/j of weights + the 384KB x8_0
            # panel) and a later bf16 pass, recombined via SBUF. The PE gets
            # ~10us of DR work that depends on only ~1.9MB of wire data, so
            # it stays busy from ~3us while the 2.5MB xb_0 panel streams.
            # sync ring: all-j fp8 weights first, then bf16 weights.
            nc.sync.dma_start(out=w8_ts[0][:, 0:2, :], in_=w8.ap()[0][:, 0:2, :])
            nc.sync.dma_start(out=w8_ts[0][:, 2:F8N, :],
                              in_=w8.ap()[0][:, 2:F8N, :])
            for j in range(1, JB):
                nc.sync.dma_start(out=w8_ts[j][:], in_=w8.ap()[j])
            for k0, k1 in ((0, 6), (6, 13), (13, NB)):
                nc.sync.dma_start(out=wb_ts[0][:, k0:k1, :],
                                  in_=wb.ap()[0][:, k0:k1, :])
            w_gate = None
            for j in range(1, JB):
                nc.sync.dma_start(out=wb_ts[j][:, 0:11, :],
                                  in_=wb.ap()[j][:, 0:11, :])
                d = nc.sync.dma_start(out=wb_ts[j][:, 11:NB, :],
                                      in_=wb.ap()[j][:, 11:NB, :])
                if j == 5:
                    w_gate = d

            # scalar ring: x8_0 (chunked, feeds the DR passes), xb_0
            # (chunked, feeds the bf16 passes), then the ti>=1 panels (xb_1
            # gated behind the weight stream).
            nc.scalar.dma_start(out=x8_ts[0][:, 0:2, :],
                                 in_=x8s[0].ap()[:, 0:2, :])
            nc.scalar.dma_start(out=x8_ts[0][:, 2:6, :],
                                 in_=x8s[0].ap()[:, 2:6, :])
            # early xb_0 slots squeeze in before the x8_0 tail: phase B's
            # start depends on them, while phase A only hits the last x8
            # pair a little later.
            nc.scalar.dma_start(out=xb_ts[0][:, 0:2, :],
                                 in_=xbs[0].ap()[:, 0:2, :])
            nc.scalar.dma_start(out=x8_ts[0][:, 6:F8N, :],
                                 in_=x8s[0].ap()[:, 6:F8N, :])
            for c0, c1 in ((2, 6), (6, 12), (12, 20)):
                nc.scalar.dma_start(out=xb_ts[0][:, c0:c1, :],
                                    in_=xbs[0].ap()[:, c0:c1, :])
            for ti in range(1, NT):
                nc.scalar.dma_start(out=x8_ts[ti][:], in_=x8s[ti].ap())
                d = nc.scalar.dma_start(out=xb_ts[ti][:], in_=xbs[ti].ap())
                if ti == 1 and w_gate is not None:
                    add_dep_helper(d.ins, w_gate.ins, sync=True,
                                   reason="w stream before x prefetch")

            # tile 0, phase A: DR-only passes, evicted to SBUF with the
            # bias fold already applied: oA = (psA + 64 b) * (1/64).
            oa_ts = []
            for j in range(JB):
                psa = pspool.tile([BLOCK, TW], mybir.dt.float32, tag="ps")
                for d in range(F8N // 2):
                    nc.tensor.matmul(psa[:],
                                     w8_ts[j][:, 2 * d:2 * d + 2, :],
                                     x8_ts[0][:, 2 * d:2 * d + 2, :],
                                     start=(d == 0), stop=(d == F8N // 2 - 1),
                                     perf_mode=_DR)
                oa = apool.tile([BLOCK, TW], mybir.dt.float32, tag="oa")
                nc.vector.tensor_scalar(
                    oa[:], psa[:], bc_t[:, j * 16:j * 16 + 1], inv,
                    op0=mybir.AluOpType.add, op1=mybir.AluOpType.mult)
                oa_ts.append(oa)

            # tile 0, phase B: bf16 passes; recombine o = psB/64 + oA.
            for j in range(JB):
                psb = pspool.tile([BLOCK, TW], mybir.dt.float32, tag="ps")
                for kk in range(NB):
                    nc.tensor.matmul(psb[:], wb_ts[j][:, kk, :],
                                     xb_ts[0][:, kk, :],
                                     start=(kk == 0), stop=(kk == NB - 1))
                o_t = opool.tile([BLOCK, TW], mybir.dt.float32, tag="o")
                nc.vector.scalar_tensor_tensor(
                    o_t[:], psb[:], inv, oa_ts[j][:],
                    op0=mybir.AluOpType.mult, op1=mybir.AluOpType.add)
                nc.gpsimd.dma_start(out=out_ap[0][j], in_=o_t[:])

            # tiles 1..3: fused 6 DR + 20 bf16 groups.
            for ti in range(1, NT):
                for j in range(JB):
                    ps = pspool.tile([BLOCK, TW], mybir.dt.float32, tag="ps")
                    for d in range(F8N // 2):
                        nc.tensor.matmul(ps[:],
                                         w8_ts[j][:, 2 * d:2 * d + 2, :],
                                         x8_ts[ti][:, 2 * d:2 * d + 2, :],
                                         start=(d == 0), stop=False,
                                         perf_mode=_DR)
                    for kk in range(NB):
                        nc.tensor.matmul(ps[:], wb_ts[j][:, kk, :],
                                         xb_ts[ti][:, kk, :],
                                         start=False, stop=(kk == NB - 1))
                    o_t = opool.tile([BLOCK, TW], mybir.dt.float32, tag="o")
                    bcol = bc_t[:, j * 16:j * 16 + 1]
                    otile = out_ap[ti][j]
                    if ti == NT - 1 and j == JB - 1:
                        # tail: halve the bias-add across engines, halve the
                        # final DMA across the two idle HWDGE rings.
                        h = TW // 2
                        nc.vector.tensor_scalar(
                            o_t[:, 0:h], ps[:, 0:h], bcol, inv,
                            op0=mybir.AluOpType.add, op1=mybir.AluOpType.mult)
                        nc.scalar.activation(
                            o_t[:, h:TW], ps[:, h:TW],
                            mybir.ActivationFunctionType.Identity,
                            bias=bc_t[:, JB * 16:JB * 16 + 1], scale=inv)
                        nc.sync.dma_start(out=otile[:, 0:h], in_=o_t[:, 0:h])
                        nc.scalar.dma_start(out=otile[:, h:TW],
                                            in_=o_t[:, h:TW])
                    else:
                        nc.vector.tensor_scalar(
                            o_t[:], ps[:], bcol, inv,
                            op0=mybir.AluOpType.add, op1=mybir.AluOpType.mult)
                        nc.gpsimd.dma_start(out=otile[:], in_=o_t[:])

    nc.compile()
    return nc


def _get_program():
    global _compiled
    if _compiled is None:
        _compiled = _build_program()
    return _compiled


def _q8(a):
    return a.astype(_F8).astype(np.float32)


def _select(x_flat, wd):
    """Cluster the 32 output-row-blocks into 4 shards and pick each shard's
    F8N fp8 k-blocks by absolute output-error cost."""
    T = x_flat.shape[0]
    wblk = wd.reshape(KB, BLOCK, KB, BLOCK)
    whi = _q8(wd * WS) / WS
    e2 = ((whi.reshape(KB, BLOCK, KB, BLOCK) - wblk) ** 2).sum(axis=(1, 3))
    n2 = (wblk ** 2).sum(axis=(1, 3))
    xq = _q8(x_flat)
    xe = xq - x_flat
    xe2_k = np.array([(xe[:, k * BLOCK:(k + 1) * BLOCK] ** 2).sum()
                      for k in range(KB)])
    cost = T * e2 + xe2_k[None, :] * n2 / BLOCK

    def cluster_once(order):
        remaining = set(range(KB))
        groups = []
        while remaining:
            seed = min(remaining,
                       key=lambda j: (np.sort(cost[j])[:F8N].sum(), order[j]))
            g = [seed]
            remaining.discard(seed)
            while len(g) < JB and remaining:
                best, bestv = None, None
                for j in remaining:
                    v = np.sort(cost[g + [j]].sum(axis=0))[:F8N].sum()
                    if bestv is None or v < bestv:
                        bestv, best = v, j
                g.append(best)
                remaining.discard(best)
            groups.append(g)
        tot = sum(np.sort(cost[g].sum(axis=0))[:F8N].sum() for g in groups)
        return tot, groups

    rng = np.random.default_rng(0)
    best = cluster_once(np.arange(KB))
    for _ in range(39):
        cand = cluster_once(rng.permutation(KB))
        if cand[0] < best[0]:
            best = cand
    groups = best[1]
    ksets = []
    for g in groups:
        c = cost[g].sum(axis=0)
        ksets.append(np.sort(np.argsort(c)[:F8N]))
    return groups, ksets


def _shard_inputs(x, weight, weight_scale_inv, bias):
    x_flat = np.ascontiguousarray(x.reshape(B * S, IN))
    wd = (weight.reshape(KB, BLOCK, KB, BLOCK)
          * weight_scale_inv[:, None, :, None]).reshape(OUT, IN)
    groups, ksets = _select(x_flat, wd)

    w8_full = (wd * WS).astype(_F8)          # [OUT, IN] fp8 (scaled)
    wb_full = (wd * WS).astype(_BF16)        # [OUT, IN] bf16 (scaled, exact)
    x8_full = x_flat.astype(_F8)             # [T, IN] fp8
    xb_full = x_flat.astype(_BF16)           # [T, IN] bf16

    in_maps = []
    for c in range(N_CORES):
        tb, gi = divmod(c, OB_SPLIT)
        ks = ksets[gi]
        kb_rest = np.array([k for k in range(KB) if k not in set(ks.tolist())])
        rows = groups[gi]                     # 8 global j-block indices

        x_sh8 = x8_full[tb * T_SH:(tb + 1) * T_SH]
        x_shb = xb_full[tb * T_SH:(tb + 1) * T_SH]
        m = {}
        for ti in range(NT):
            sl = slice(ti * TW, (ti + 1) * TW)
            # pan[p, slot, t] = x[ti*TW+t, k_slot*128+p]
            p8 = x_sh8[sl].reshape(TW, KB, BLOCK)[:, ks, :].transpose(2, 1, 0)
            pb = x_shb[sl].reshape(TW, KB, BLOCK)[:, kb_rest, :] \
                .transpose(2, 1, 0)
            m[f"x8_{ti}"] = np.ascontiguousarray(p8)
            m[f"xb_{ti}"] = np.ascontiguousarray(pb)

        w8v = np.empty((JB, BLOCK, F8N, BLOCK), dtype=_F8)
        wbv = np.empty((JB, BLOCK, NB, BLOCK), dtype=_BF16)
        bcv = np.empty((BLOCK, (JB + 1) * 16), dtype=np.float32)
        for jj, gj in enumerate(rows):
            rsl = slice(gj * BLOCK, (gj + 1) * BLOCK)
            # wt[p, slot, o] = w[gj*128+o, k_slot*128+p]
            w8v[jj] = w8_full[rsl].reshape(BLOCK, KB, BLOCK)[:, ks, :] \
                .transpose(2, 1, 0)
            wbv[jj] = wb_full[rsl].reshape(BLOCK, KB, BLOCK)[:, kb_rest, :] \
                .transpose(2, 1, 0)
            bcv[:, jj * 16:(jj + 1) * 16] = \
                (WS * bias[rsl])[:, None].astype(np.float32)
            if jj == JB - 1:
                bcv[:, JB * 16:] = bias[rsl][:, None].astype(np.float32)
        m["w8"] = np.ascontiguousarray(w8v)
        m["wb"] = np.ascontiguousarray(wbv)
        m["bc"] = bcv
        in_maps.append(m)
    return in_maps, groups


def _run(in_maps, trace=False):
    nc = _get_program()
    return run_bass_kernel_spmd(nc, in_maps, list(range(N_CORES)),
                                trace=trace)


def _assemble(results, groups):
    out_full = np.empty((B * S, OUT), dtype=np.float32)
    for c in range(N_CORES):
        tb, gi = divmod(c, OB_SPLIT)
        out_c = np.asarray(results[c]["out"], dtype=np.float32)
        # [NT, JB, BLOCK, TW] -> [T_SH, JB*BLOCK]
        out_c = out_c.transpose(0, 3, 1, 2).reshape(T_SH, O_SH)
        for jj, gj in enumerate(groups[gi]):
            out_full[tb * T_SH:(tb + 1) * T_SH,
                     gj * BLOCK:(gj + 1) * BLOCK] = \
                out_c[:, jj * BLOCK:(jj + 1) * BLOCK]
    return out_full.reshape(B, S, OUT)


def kernel(x, weight, weight_scale_inv, bias):
    x = np.asarray(x, dtype=np.float32)
    weight = np.asarray(weight, dtype=np.float32)
    weight_scale_inv = np.asarray(weight_scale_inv, dtype=np.float32)
    bias = np.asarray(bias, dtype=np.float32)
    assert x.shape == (B, S, IN), x.shape
    assert weight.shape == (OUT, IN), weight.shape
    assert weight_scale_inv.shape == (OUT // BLOCK, IN // BLOCK)
    assert bias.shape == (OUT,)

    in_maps, groups = _shard_inputs(x, weight, weight_scale_inv, bias)
    res = _run(in_maps)
    return _assemble(res.results, groups)


# revision 15
# speedup vs baseline: 1.1209x; 1.1209x over previous
"""Blocked-FP8 linear (dequant + matmul + bias) on 8 Trainium2 NeuronCores.

Computation: out[b,s,o] = sum_i x[b,s,i] * (weight[o,i] * scale_inv[o//128, i//128]) + bias[o]
Shapes: x [2, 2048, 4096] f32, weight [4096, 4096] f32 (e4m3-quantized values),
        weight_scale_inv [32, 32] f32, bias [4096] f32 -> out [2, 2048, 4096] f32.

Sharding: 2-way over tokens x 4-way over out_features (colwise tensor-parallel,
no collectives). Each core computes a [2048 token, 1024 out] block as
out.T = W_deq @ X.T with K(=in_features) on the partition dim.

Speed scheme (v2): the PE streams the moving operand at 1 column/cycle, so a
bf16 [128k x 512tok] matmul costs ~216 ns regardless of dtype. The only 2x
lever is fp8 DoubleRow (both operands fp8e4): one instruction contracts TWO
128-k-blocks in one 1024-column sweep (~231 ns measured, 1.87x per block).
Per core, F=10 of the 32 k-blocks ride fp8 DoubleRow; the rest stay bf16.
Per-(ti,j) group: 5 DR + 22 bf16 matmuls into one PSUM bank.

Accuracy budget (rel tol 2e-2): an fp8 block pays w-requant (~2.6e-2 rel,
block-dependent via the scale mantissa) + x-fp8 (~2.65e-2) on its slice.
Three tricks keep the total at ~1.79e-2 (sim matches HW to 4 digits):
 1. weights are pre-scaled by 64 before e4m3 quantization (values ~0.02 sit
    in e4m3's denormal zone otherwise; x64 moves them to the normal range).
    All weights (bf16 too, exact there) carry the x64; the bias step computes
    out = (psum + 64*bias) * (1/64) in one fused DVE tensor_scalar.
 2. block selection: which k-blocks go fp8 is chosen per core from the
    actual absolute error cost T*||q8(64 w_blk)-64 w_blk||^2/4096 +
    ||x_err(k)||^2*||w_blk||^2/128, summed over the core's 8 j-rows.
 3. row clustering: the assignment of the 32 global output-row-blocks to the
    4 column-shards is chosen (greedy) so rows sharing low-cost k-blocks land
    in the same shard; the host remaps rows on assemble.

Schedule: startup is HBM-bound. Rings: sync=weights (fp8 j0 first so the
DR-first group order can start on ~190KB), scalar=xb bf16 panels (chunked,
ti>=1 gated behind the weight stream), vector=x8 fp8 panels (small, land
first), gpsimd=consts+outputs (even j; odd j outputs ride the vector ring).
The final (ti=3,j=7) group splits its bias-add across vector/gpsimd halves
and its output DMA across four rings to shorten the tail drain.
"""

import os
import sys

for _p in ("/opt/trn_rl_repo", "/root/.axon_site/_ro/trn_rl_repo"):
    if os.path.isdir(_p) and _p not in sys.path:
        sys.path.insert(0, _p)

import ml_dtypes
import numpy as np

import concourse.bass as bass  # noqa: F401  (registers AP machinery)
import concourse.tile as tile
from concourse import bacc, mybir
from concourse.bass_utils import run_bass_kernel_spmd
from concourse.tile import add_dep_helper

BLOCK = 128
B, S, IN, OUT = 2, 2048, 4096, 4096
N_CORES = 8
TB_SPLIT = 2            # token split
OB_SPLIT = 4            # out_features split (shard = clustered row group)
T_SH = B * S // TB_SPLIT    # 2048 tokens per core
O_SH = OUT // OB_SPLIT      # 1024 out features per core
KB = IN // BLOCK            # 32 k-blocks
JB = O_SH // BLOCK          # 8 local o-blocks
NT = 4                      # token tiles per core
TW = 512                    # tokens per tile (PSUM bank width in fp32)
F8N = 12                    # fp8 DoubleRow k-blocks per core (6 DR pairs)
NB = KB - F8N               # bf16 k-blocks
WS = 64.0                   # weight pre-scale (out of e4m3 denormal zone)
N_WARM = 52                 # PE clock warm-up matmuls

_BF16 = ml_dtypes.bfloat16
_F8 = ml_dtypes.float8_e4m3   # TRN float8e4 (IEEE-ish, max 240)
_DR = mybir.MatmulPerfMode.DoubleRow

_compiled = None


def _build_program():
    nc = bacc.Bacc("TRN2", target_bir_lowering=False, debug=False,
                   num_devices=N_CORES)

    x8s = [nc.dram_tensor(f"x8_{ti}", [BLOCK, F8N, TW], mybir.dt.float8e4,
                          kind="ExternalInput") for ti in range(NT)]
    xbs = [nc.dram_tensor(f"xb_{ti}", [BLOCK, NB, TW], mybir.dt.bfloat16,
                          kind="ExternalInput") for ti in range(NT)]
    w8 = nc.dram_tensor("w8", [JB, BLOCK, F8N, BLOCK], mybir.dt.float8e4,
                        kind="ExternalInput")
    wb = nc.dram_tensor("wb", [JB, BLOCK, NB, BLOCK], mybir.dt.bfloat16,
                        kind="ExternalInput")
    # bias columns (x64, to fold into (psum + 64 b) * (1/64)), padded 16x;
    # final 16 cols hold the UNSCALED bias of the last j-block for the
    # scalar-engine activation (out = Identity(psum * 1/64 + b)) in the tail.
    bc = nc.dram_tensor("bc", [BLOCK, (JB + 1) * 16], mybir.dt.float32,
                        kind="ExternalInput")
    # contiguous per-(ti,j) tiles: strided 2KB-row writes into a [O_SH,T_SH]
    # matrix drain at ~60 GB/s; a contiguous 256KB block is much faster.
    out = nc.dram_tensor("out", [NT, JB, BLOCK, TW], mybir.dt.float32,
                         kind="ExternalOutput")

    out_ap = out.ap()
    inv = 1.0 / WS

    with tile.TileContext(nc) as tc:
        with (
            tc.tile_pool(name="consts", bufs=1) as consts,
            tc.tile_pool(name="w8pool", bufs=JB) as w8pool,
            tc.tile_pool(name="wbpool", bufs=JB) as wbpool,
            tc.tile_pool(name="x8pool", bufs=NT) as x8pool,
            tc.tile_pool(name="xbpool", bufs=NT) as xbpool,
            tc.tile_pool(name="opool", bufs=8) as opool,
            tc.tile_pool(name="apool", bufs=JB) as apool,
            tc.tile_pool(name="pspool", bufs=7, space="PSUM") as pspool,
            tc.tile_pool(name="warmps", bufs=1, space="PSUM") as warmps,
        ):
            # PE warm-up on a zeroed scratch tile: keeps the PE clock ramp
            # (HAM) running so the first real matmuls execute at full rate.
            warm = consts.tile([BLOCK, BLOCK], mybir.dt.bfloat16)
            nc.gpsimd.memset(warm[:], 0.0)
            ps_warm = warmps.tile([BLOCK, BLOCK], mybir.dt.float32)
            for _ in range(N_WARM):
                nc.tensor.matmul(ps_warm[:], warm[:], warm[:],
                                 start=True, stop=True)

            bc_t = consts.tile([BLOCK, (JB + 1) * 16], mybir.dt.float32)
            nc.gpsimd.dma_start(out=bc_t[:], in_=bc.ap())

            w8_ts = [w8pool.tile([BLOCK, F8N, BLOCK], mybir.dt.float8e4,
                                 name=f"w8_{j}", tag="w8") for j in range(JB)]
            wb_ts = [wbpool.tile([BLOCK, NB, BLOCK], mybir.dt.bfloat16,
                                 name=f"wb_{j}", tag="wb") for j in range(JB)]
            x8_ts = [x8pool.tile([BLOCK, F8N, TW], mybir.dt.float8e4,
                                 name=f"x8t_{ti}", tag="x8") for ti in range(NT)]
            xb_ts = [xbpool.tile([BLOCK, NB, TW], mybir.dt.bfloat16,
                                 name=f"xbt_{ti}", tag="xb") for ti in range(NT)]

            # Startup plan: tile-0 is processed in TWO passes per j-group —
            # a DR-only pass (all fp8: ~190KB/j of weights + the 384KB x8_0
            # panel) and a later bf16 pass, recombined via SBUF. The PE gets
            # ~10us of DR work that depends on only ~1.9MB of wire data, so
            # it stays busy from ~3us while the 2.5MB xb_0 panel streams.
            # sync ring: all-j fp8 weights first, then bf16 weights.
            nc.sync.dma_start(out=w8_ts[0][:, 0:2, :], in_=w8.ap()[0][:, 0:2, :])
            nc.sync.dma_start(out=w8_ts[0][:, 2:F8N, :],
                              in_=w8.ap()[0][:, 2:F8N, :])
            for j in range(1, JB):
                nc.sync.dma_start(out=w8_ts[j][:], in_=w8.ap()[j])
            for k0, k1 in ((0, 6), (6, 13), (13, NB)):
                nc.sync.dma_start(out=wb_ts[0][:, k0:k1, :],
                                  in_=wb.ap()[0][:, k0:k1, :])
            w_gate = None
            for j in range(1, JB):
                nc.sync.dma_start(out=wb_ts[j][:, 0:11, :],
                                  in_=wb.ap()[j][:, 0:11, :])
                d = nc.sync.dma_start(out=wb_ts[j][:, 11:NB, :],
                                      in_=wb.ap()[j][:, 11:NB, :])
                if j == 5:
                    w_gate = d

            # scalar ring: x8_0 (chunked, feeds the DR passes), xb_0
            # (chunked, feeds the bf16 passes), then the ti>=1 panels (xb_1
            # gated behind the weight stream).
            nc.scalar.dma_start(out=x8_ts[0][:, 0:2, :],
                                 in_=x8s[0].ap()[:, 0:2, :])
            nc.scalar.dma_start(out=x8_ts[0][:, 2:6, :],
                                 in_=x8s[0].ap()[:, 2:6, :])
            # early xb_0 slots squeeze in before the x8_0 tail: phase B's
            # start depends on them, while phase A only hits the last x8
            # pair a little later.
            nc.scalar.dma_start(out=xb_ts[0][:, 0:2, :],
                                 in_=xbs[0].ap()[:, 0:2, :])
            nc.scalar.dma_start(out=x8_ts[0][:, 6:F8N, :],
                                 in_=x8s[0].ap()[:, 6:F8N, :])
            for c0, c1 in ((2, 6), (6, 12), (12, 20)):
                nc.scalar.dma_start(out=xb_ts[0][:, c0:c1, :],
                                    in_=xbs[0].ap()[:, c0:c1, :])
            for ti in range(1, NT):
                nc.scalar.dma_start(out=x8_ts[ti][:], in_=x8s[ti].ap())
                d = nc.scalar.dma_start(out=xb_ts[ti][:], in_=xbs[ti].ap())
                if ti == 1 and w_gate is not None:
                    add_dep_helper(d.ins, w_gate.ins, sync=True,
                                   reason="w stream before x prefetch")

            # tile 0, phase A: DR-only passes, evicted to SBUF with the
            # bias fold already applied: oA = (psA + 64 b) * (1/64).
            oa_ts = []
            for j in range(JB):
                psa = pspool.tile([BLOCK, TW], mybir.dt.float32, tag="ps")
                for d in range(F8N // 2):
                    nc.tensor.matmul(psa[:],
                                     w8_ts[j][:, 2 * d:2 * d + 2, :],
                                     x8_ts[0][:, 2 * d:2 * d + 2, :],
                                     start=(d == 0), stop=(d == F8N // 2 - 1),
                                     perf_mode=_DR)
                oa = apool.tile([BLOCK, TW], mybir.dt.float32, tag="oa")
                nc.vector.tensor_scalar(
                    oa[:], psa[:], bc_t[:, j * 16:j * 16 + 1], inv,
                    op0=mybir.AluOpType.add, op1=mybir.AluOpType.mult)
                oa_ts.append(oa)

            # tile 0, phase B: bf16 passes; recombine o = psB/64 + oA.
            for j in range(JB):
                psb = pspool.tile([BLOCK, TW], mybir.dt.float32, tag="ps")
                for kk in range(NB):
                    nc.tensor.matmul(psb[:], wb_ts[j][:, kk, :],
                                     xb_ts[0][:, kk, :],
                                     start=(kk == 0), stop=(kk == NB - 1))
                o_t = opool.tile([BLOCK, TW], mybir.dt.float32, tag="o")
                nc.vector.scalar_tensor_tensor(
                    o_t[:], psb[:], inv, oa_ts[j][:],
                    op0=mybir.AluOpType.mult, op1=mybir.AluOpType.add)
                nc.gpsimd.dma_start(out=out_ap[0][j], in_=o_t[:])

            # tiles 1..3: fused 6 DR + 20 bf16 groups.
            for ti in range(1, NT):
                for j in range(JB):
                    ps = pspool.tile([BLOCK, TW], mybir.dt.float32, tag="ps")
                    for d in range(F8N // 2):
                        nc.tensor.matmul(ps[:],
                                         w8_ts[j][:, 2 * d:2 * d + 2, :],
                                         x8_ts[ti][:, 2 * d:2 * d + 2, :],
                                         start=(d == 0), stop=False,
                                         perf_mode=_DR)
                    for kk in range(NB):
                        nc.tensor.matmul(ps[:], wb_ts[j][:, kk, :],
                                         xb_ts[ti][:, kk, :],
                                         start=False, stop=(kk == NB - 1))
                    o_t = opool.tile([BLOCK, TW], mybir.dt.float32, tag="o")
                    bcol = bc_t[:, j * 16:j * 16 + 1]
                    otile = out_ap[ti][j]
                    if ti == NT - 1 and j == JB - 1:
                        # tail: halve the bias-add across engines, halve the
                        # final DMA across the two idle HWDGE rings.
                        h = TW // 2
                        nc.vector.tensor_scalar(
                            o_t[:, 0:h], ps[:, 0:h], bcol, inv,
                            op0=mybir.AluOpType.add, op1=mybir.AluOpType.mult)
                        nc.scalar.activation(
                            o_t[:, h:TW], ps[:, h:TW],
                            mybir.ActivationFunctionType.Identity,
                            bias=bc_t[:, JB * 16:JB * 16 + 1], scale=inv)
                        nc.sync.dma_start(out=otile[:, 0:h], in_=o_t[:, 0:h])
                        nc.scalar.dma_start(out=otile[:, h:TW],
                                            in_=o_t[:, h:TW])
                    else:
                        nc.vector.tensor_scalar(
                            o_t[:], ps[:], bcol, inv,
                            op0=mybir.AluOpType.add, op1=mybir.AluOpType.mult)
                        nc.gpsimd.dma_start(out=otile[:], in_=o_t[:])

    nc.compile()
    return nc


def _get_program():
    global _compiled
    if _compiled is None:
        _compiled = _build_program()
    return _compiled


def _q8(a):
    return a.astype(_F8).astype(np.float32)


def _select(x_flat, wd):
    """Cluster the 32 output-row-blocks into 4 shards and pick each shard's
    F8N fp8 k-blocks by absolute output-error cost."""
    T = x_flat.shape[0]
    wblk = wd.reshape(KB, BLOCK, KB, BLOCK)
    whi = _q8(wd * WS) / WS
    e2 = ((whi.reshape(KB, BLOCK, KB, BLOCK) - wblk) ** 2).sum(axis=(1, 3))
    n2 = (wblk ** 2).sum(axis=(1, 3))
    xq = _q8(x_flat)
    xe = xq - x_flat
    xe2_k = np.array([(xe[:, k * BLOCK:(k + 1) * BLOCK] ** 2).sum()
                      for k in range(KB)])
    cost = T * e2 + xe2_k[None, :] * n2 / BLOCK

    def cluster_once(order):
        remaining = set(range(KB))
        groups = []
        while remaining:
            seed = min(remaining,
                       key=lambda j: (np.sort(cost[j])[:F8N].sum(), order[j]))
            g = [seed]
            remaining.discard(seed)
            while len(g) < JB and remaining:
                best, bestv = None, None
                for j in remaining:
                    v = np.sort(cost[g + [j]].sum(axis=0))[:F8N].sum()
                    if bestv is None or v < bestv:
                        bestv, best = v, j
                g.append(best)
                remaining.discard(best)
            groups.append(g)
        tot = sum(np.sort(cost[g].sum(axis=0))[:F8N].sum() for g in groups)
        return tot, groups

    rng = np.random.default_rng(0)
    best = cluster_once(np.arange(KB))
    for _ in range(39):
        cand = cluster_once(rng.permutation(KB))
        if cand[0] < best[0]:
            best = cand
    groups = best[1]
    ksets = []
    for g in groups:
        c = cost[g].sum(axis=0)
        ksets.append(np.sort(np.argsort(c)[:F8N]))
    return groups, ksets


def _shard_inputs(x, weight, weight_scale_inv, bias):
    x_flat = np.ascontiguousarray(x.reshape(B * S, IN))
    wd = (weight.reshape(KB, BLOCK, KB, BLOCK)
          * weight_scale_inv[:, None, :, None]).reshape(OUT, IN)
    groups, ksets = _select(x_flat, wd)

    w8_full = (wd * WS).astype(_F8)          # [OUT, IN] fp8 (scaled)
    wb_full = (wd * WS).astype(_BF16)        # [OUT, IN] bf16 (scaled, exact)
    x8_full = x_flat.astype(_F8)             # [T, IN] fp8
    xb_full = x_flat.astype(_BF16)           # [T, IN] bf16

    in_maps = []
    for c in range(N_CORES):
        tb, gi = divmod(c, OB_SPLIT)
        ks = ksets[gi]
        kb_rest = np.array([k for k in range(KB) if k not in set(ks.tolist())])
        rows = groups[gi]                     # 8 global j-block indices

        x_sh8 = x8_full[tb * T_SH:(tb + 1) * T_SH]
        x_shb = xb_full[tb * T_SH:(tb + 1) * T_SH]
        m = {}
        for ti in range(NT):
            sl = slice(ti * TW, (ti + 1) * TW)
            # pan[p, slot, t] = x[ti*TW+t, k_slot*128+p]
            p8 = x_sh8[sl].reshape(TW, KB, BLOCK)[:, ks, :].transpose(2, 1, 0)
            pb = x_shb[sl].reshape(TW, KB, BLOCK)[:, kb_rest, :] \
                .transpose(2, 1, 0)
            m[f"x8_{ti}"] = np.ascontiguousarray(p8)
            m[f"xb_{ti}"] = np.ascontiguousarray(pb)

        w8v = np.empty((JB, BLOCK, F8N, BLOCK), dtype=_F8)
        wbv = np.empty((JB, BLOCK, NB, BLOCK), dtype=_BF16)
        bcv = np.empty((BLOCK, (JB + 1) * 16), dtype=np.float32)
        for jj, gj in enumerate(rows):
            rsl = slice(gj * BLOCK, (gj + 1) * BLOCK)
            # wt[p, slot, o] = w[gj*128+o, k_slot*128+p]
            w8v[jj] = w8_full[rsl].reshape(BLOCK, KB, BLOCK)[:, ks, :] \
                .transpose(2, 1, 0)
            wbv[jj] = wb_full[rsl].reshape(BLOCK, KB, BLOCK)[:, kb_rest, :] \
                .transpose(2, 1, 0)
            bcv[:, jj * 16:(jj + 1) * 16] = \
                (WS * bias[rsl])[:, None].astype(np.float32)
            if jj == JB - 1:
                bcv[:, JB * 16:] = bias[rsl][:, None].astype(np.float32)
        m["w8"] = np.ascontiguousarray(w8v)
        m["wb"] = np.ascontiguousarray(wbv)
        m["bc"] = bcv
        in_maps.append(m)
    return in_maps, groups


def _run(in_maps, trace=False):
    nc = _get_program()
    return run_bass_kernel_spmd(nc, in_maps, list(range(N_CORES)),
                                trace=trace)


def _assemble(results, groups):
    out_full = np.empty((B * S, OUT), dtype=np.float32)
    for c in range(N_CORES):
        tb, gi = divmod(c, OB_SPLIT)
        out_c = np.asarray(results[c]["out"], dtype=np.float32)
        # [NT, JB, BLOCK, TW] -> [T_SH, JB*BLOCK]
        out_c = out_c.transpose(0, 3, 1, 2).reshape(T_SH, O_SH)
        for jj, gj in enumerate(groups[gi]):
            out_full[tb * T_SH:(tb + 1) * T_SH,
                     gj * BLOCK:(gj + 1) * BLOCK] = \
                out_c[:, jj * BLOCK:(jj + 1) * BLOCK]
    return out_full.reshape(B, S, OUT)


def kernel(x, weight, weight_scale_inv, bias):
    x = np.asarray(x, dtype=np.float32)
    weight = np.asarray(weight, dtype=np.float32)
    weight_scale_inv = np.asarray(weight_scale_inv, dtype=np.float32)
    bias = np.asarray(bias, dtype=np.float32)
    assert x.shape == (B, S, IN), x.shape
    assert weight.shape == (OUT, IN), weight.shape
    assert weight_scale_inv.shape == (OUT // BLOCK, IN // BLOCK)
    assert bias.shape == (OUT,)

    in_maps, groups = _shard_inputs(x, weight, weight_scale_inv, bias)
    res = _run(in_maps)
    return _assemble(res.results, groups)
